# revision 12
# baseline (speedup 1.0000x reference)
"""Grouped SwiGLU MoE MLP (16 experts) on 8 NeuronCores, expert-parallel.

Reference computation, per expert e over its contiguous token slice xi:
    out = (silu(xi @ w_gate[e].T) * (xi @ w_up[e].T)) @ w_down[e].T

Sharding: expert-parallel. Core c owns experts {2c, 2c+1}; the host hands it
the matching contiguous 2048-token slice of x (tokens are pre-sorted by
expert), so no device-side collectives are needed.

All matmul operands are bf16 (host-cast): the PE streaming rate matches
fp32r but FWL halves LDWEIGHTS time and input DMA bytes halve. PSUM
accumulation stays fp32 and the output is fp32 (~4e-3 rel err, inside the
2e-2 gate).

Layout: pre-packed on the host so every DMA is a contiguous block:
  xq  [EPC, P, HT, TPE]   xq[e,p,a,t]  = x[e*TPE+t, a*P+p]
  wgq [EPC, FT, P, HT*P]  wgq[e,f,p,a*P+j] = w_gate[e, f*P+j, a*P+p]
  wuq  same as wgq for w_up
  wdq [EPC, HG, P, FT*P]  wdq[e,g,p,f*P+j] = w_down[e, g*P+j, f*P+p]
  outq[EPC, HG, P, TPE]   outq[e,g,p,t] = out[e*TPE+t, g*P+p]

Schedule (from trace analysis of earlier revisions):
- The framework preamble ends ~6.5us and the first DMA packet lands ~8us;
  NWARM dummy matmuls on a zeroed tile hold HAM at K=8/8 through that
  window so real work starts at 2.4GHz.
- Chains are t-half-outer so the first f-tile only needs the th=0 half of
  x (1MB instead of 2MB) before the PE can run 32 back-to-back matmuls,
  and so each t-half's PSUM pair drains while the other half's chains run.
- The Scalar queue runs ONLY the silu ACTIVATEs: a dma_start whose
  pool-pacing semaphore isn't yet satisfied parks its whole queue, and
  parking the silu queue delays PSUM consumption and costs the PE one
  matmul slot per group. All steady-state DMA goes on the Sync ring,
  ordered so no paced load ever sits ahead of a store it would block.
- Expert 0's first-tile weights + x are split across both rings for
  startup bandwidth (the Scalar queue is empty until the first silu).
"""

import numpy as np
import ml_dtypes

import concourse.bass as bass
import concourse.bacc as bacc
import concourse.mybir as mybir
from concourse import tile
from concourse.bass_utils import run_bass_kernel_spmd

E, T, H, F = 16, 16384, 1024, 2048
NCORES = 8
EPC = E // NCORES          # experts per core
TPE = T // E               # tokens per expert (uniform fast path)
P = 128                    # SBUF partitions
HT = H // P                # 8 h-tiles (contraction tiles for gate/up)
FT = F // P                # 16 f-tiles
HGS = H // P               # 8 output h-groups for down proj
NT = 512                   # matmul moving free dim (PSUM bank = 512 fp32)
TH = TPE // NT             # 2 t-halves
NWARM = 36                 # N=128 dummy matmuls to pre-warm the PE clock

BF16 = mybir.dt.bfloat16
F32 = mybir.dt.float32
BF16_NP = ml_dtypes.bfloat16

_CACHE = {}

# Set by run for test harness introspection (exec_time_ns, profile).
LAST_RESULTS = None
TRACE = False
TRACE_KW = {}


def _build_nc():
    nc = bacc.Bacc()
    xq = nc.dram_tensor("xq", [EPC, P, HT, TPE], BF16, kind="ExternalInput")
    wgq = nc.dram_tensor("wgq", [EPC, FT, P, HT * P], BF16, kind="ExternalInput")
    wuq = nc.dram_tensor("wuq", [EPC, FT, P, HT * P], BF16, kind="ExternalInput")
    wdq = nc.dram_tensor("wdq", [EPC, HGS, P, FT * P], BF16, kind="ExternalInput")
    outq = nc.dram_tensor("outq", [EPC, HGS, P, TPE], F32, kind="ExternalOutput")

    with tile.TileContext(nc) as tc:
        with (
            tc.tile_pool(name="xp", bufs=2) as xp,
            tc.tile_pool(name="wgp", bufs=5) as wgp,
            tc.tile_pool(name="wup", bufs=5) as wup,
            tc.tile_pool(name="wdp", bufs=8) as wdp,
            tc.tile_pool(name="hid", bufs=FT + 1) as hidp,
            tc.tile_pool(name="tmp", bufs=5) as tmpp,
            tc.tile_pool(name="osb", bufs=4) as osbp,
            tc.tile_pool(name="ps", bufs=8, space=bass.MemorySpace.PSUM) as psp,
        ):
            # PE warm-up (see module docstring). N=128 dummies burn the
            # HAM cold window (~3.4us at 1.2GHz) in fine 107ns steps so the
            # bridge ends close to when the startup DMA lands; the memset
            # runs on the DVE, which is idle until the first mul.
            warm = tmpp.tile([P, P], BF16, tag="warm", name="warm")
            nc.vector.memset(warm[:], 0.0)
            wps = psp.tile([P, NT], F32, tag="ps", name="warm_ps")
            for _ in range(NWARM):
                nc.tensor.matmul(wps[:, 0:P], warm[:], warm[:],
                                 start=True, stop=True)

            xts = {}
            wgts = {}
            wuts = {}

            def prefetch_head(el):
                """First f-tile weights + x. For expert 0 this is the
                startup-critical burst and is split across both rings; the
                Scalar queue is otherwise empty until the first silu."""
                xt = xp.tile([P, HT, TPE], BF16, tag="xt", name=f"x{el}")
                xts[el] = xt
                wgt = wgp.tile([P, HT * P], BF16, tag="wg", name=f"wg{el}_0")
                nc.sync.dma_start(wgt[:], wgq[el, 0])
                wgts[(el, 0)] = wgt
                # Startup-critical order: the gate chain needs wg0 + all of
                # x's th=0 half; wu is consumed 8 matmuls later and th=1 16
                # later. Scalar carries only x so it finishes th=0 early.
                nc.scalar.dma_start(xt[:, 4:8, 0:NT], xq[el][:, 4:8, 0:NT])
                nc.sync.dma_start(xt[:, 0:4, 0:NT], xq[el][:, 0:4, 0:NT])
                wut = wup.tile([P, HT * P], BF16, tag="wu", name=f"wu{el}_0")
                nc.sync.dma_start(wut[:], wuq[el, 0])
                wuts[(el, 0)] = wut
                nc.scalar.dma_start(xt[:, 4:8, NT:TPE], xq[el][:, 4:8, NT:TPE])
                nc.sync.dma_start(xt[:, 0:4, NT:TPE], xq[el][:, 0:4, NT:TPE])

            prefetch_head(0)
            for el in range(EPC):
                xt = xts[el]
                for ft in range(1, FT):
                    wgt = wgp.tile([P, HT * P], BF16, tag="wg", name=f"wg{el}_{ft}")
                    nc.sync.dma_start(wgt[:], wgq[el, ft])
                    wgts[(el, ft)] = wgt
                    wut = wup.tile([P, HT * P], BF16, tag="wu", name=f"wu{el}_{ft}")
                    nc.sync.dma_start(wut[:], wuq[el, ft])
                    wuts[(el, ft)] = wut
                wdts = {}
                for hg in range(HGS):
                    wdt = wdp.tile([P, FT * P], BF16, tag="wd", name=f"wd{el}_{hg}")
                    nc.sync.dma_start(wdt[:], wdq[el, hg])
                    wdts[hg] = wdt

                # Gate/up, t-half-outer: for each f-tile, run the gate and
                # up chains for th=0 (16 MMs), whose silu+mul drain while
                # the th=1 chains (16 MMs) run.
                hids = []
                for ft in range(FT):
                    wgt, wut = wgts.pop((el, ft)), wuts.pop((el, ft))
                    hid = hidp.tile([P, TPE], BF16, tag="hid", name=f"hid{el}_{ft}")
                    for th in range(TH):
                        tsl = slice(th * NT, (th + 1) * NT)
                        g_ps = psp.tile([P, NT], F32, tag="ps", name=f"g{el}_{ft}_{th}")
                        u_ps = psp.tile([P, NT], F32, tag="ps", name=f"u{el}_{ft}_{th}")
                        for ht in range(HT):
                            nc.tensor.matmul(
                                g_ps[:], wgt[:, ht * P:(ht + 1) * P],
                                xt[:, ht, tsl],
                                start=(ht == 0), stop=(ht == HT - 1),
                            )
                        for ht in range(HT):
                            nc.tensor.matmul(
                                u_ps[:], wut[:, ht * P:(ht + 1) * P],
                                xt[:, ht, tsl],
                                start=(ht == 0), stop=(ht == HT - 1),
                            )
                        tmp = tmpp.tile([P, NT], BF16, tag="tmp")
                        nc.scalar.activation(
                            tmp[:], g_ps[:],
                            mybir.ActivationFunctionType.Silu,
                        )
                        nc.vector.tensor_mul(hid[:, tsl], tmp[:], u_ps[:])
                    hids.append(hid)

                # Next expert's head prefetch goes out before this expert's
                # output stores so its x/weights are resident at the
                # expert boundary.
                if el + 1 < EPC:
                    prefetch_head(el + 1)

                # Down projection, t-half-outer: th=0's 16-MM chain
                # completes before th=1's starts, so its copy+store overlap
                # the th=1 chain and the post-loop tail is one store deep.
                for hg in range(HGS):
                    wdt = wdts[hg]
                    for th in range(TH):
                        tsl = slice(th * NT, (th + 1) * NT)
                        if el == EPC - 1 and hg == HGS - 1 and th == TH - 1:
                            # Final t-half: two N=256 chains into SEPARATE
                            # PSUM banks (a shared bank would make the first
                            # chain's copy collide with the second chain's
                            # writes), so the first quarter's store overlaps
                            # the second chain and the post-loop tail is one
                            # 64KB-store deep per queue.
                            NQ = NT // 2
                            d_qs = [psp.tile([P, NT], F32, tag="ps",
                                             name=f"d{el}_{hg}_{th}_{q}")
                                    for q in range(2)]
                            osb = osbp.tile([P, NT], F32, tag="osb")
                            for q in range(2):
                                qsl = slice(q * NQ, (q + 1) * NQ)
                                for ft in range(FT):
                                    nc.tensor.matmul(
                                        d_qs[q][:, 0:NQ],
                                        wdt[:, ft * P:(ft + 1) * P],
                                        hids[ft][:, th * NT + q * NQ:
                                                  th * NT + (q + 1) * NQ],
                                        start=(ft == 0), stop=(ft == FT - 1),
                                    )
                                if q == 0:
                                    nc.vector.tensor_copy(osb[:, qsl],
                                                          d_qs[0][:, 0:NQ])
                                    nc.scalar.dma_start(
                                        outq[el, hg][:, th * NT:th * NT + NQ],
                                        osb[:, qsl])
                            nc.vector.tensor_copy(osb[:, NQ:NQ + NQ // 2],
                                                  d_qs[1][:, 0:NQ // 2])
                            nc.scalar.activation(
                                osb[:, NQ + NQ // 2:NT],
                                d_qs[1][:, NQ // 2:NQ],
                                mybir.ActivationFunctionType.Copy,
                            )
                            nc.sync.dma_start(
                                outq[el, hg][:, th * NT + NQ:
                                             th * NT + NQ + NQ // 2],
                                osb[:, NQ:NQ + NQ // 2])
                            nc.scalar.dma_start(
                                outq[el, hg][:, th * NT + NQ + NQ // 2:
                                             (th + 1) * NT],
                                osb[:, NQ + NQ // 2:NT])
                        else:
                            d_ps = psp.tile([P, NT], F32, tag="ps",
                                            name=f"d{el}_{hg}_{th}")
                            for ft in range(FT):
                                nc.tensor.matmul(
                                    d_ps[:], wdt[:, ft * P:(ft + 1) * P],
                                    hids[ft][:, tsl],
                                    start=(ft == 0), stop=(ft == FT - 1),
                                )
                            osb = osbp.tile([P, NT], F32, tag="osb")
                            nc.vector.tensor_copy(osb[:], d_ps[:])
                            nc.sync.dma_start(outq[el, hg][:, tsl], osb[:])
    return nc


def get_nc():
    if "nc" not in _CACHE:
        nc = _build_nc()
        nc.finalize()
        _CACHE["nc"] = nc
    return _CACHE["nc"]


def make_in_maps(x, w_gate, w_up, w_down):
    xb = x.astype(BF16_NP)
    wgb = w_gate.astype(BF16_NP)
    wub = w_up.astype(BF16_NP)
    wdb = w_down.astype(BF16_NP)
    in_maps = []
    for c in range(NCORES):
        e0 = c * EPC
        # xq[e,p,a,t] = x[e*TPE+t, a*P+p]
        xs = xb[e0 * TPE:(e0 + EPC) * TPE].reshape(EPC, TPE, HT, P)
        xqc = np.ascontiguousarray(xs.transpose(0, 3, 2, 1))
        # wgq[e,f,p,a*P+j] = w_gate[e, f*P+j, a*P+p]
        wg = wgb[e0:e0 + EPC].reshape(EPC, FT, P, HT, P)
        wgc = np.ascontiguousarray(wg.transpose(0, 1, 4, 3, 2)).reshape(
            EPC, FT, P, HT * P)
        wu = wub[e0:e0 + EPC].reshape(EPC, FT, P, HT, P)
        wuc = np.ascontiguousarray(wu.transpose(0, 1, 4, 3, 2)).reshape(
            EPC, FT, P, HT * P)
        # wdq[e,g,p,f*P+j] = w_down[e, g*P+j, f*P+p]
        wd = wdb[e0:e0 + EPC].reshape(EPC, HGS, P, FT, P)
        wdc = np.ascontiguousarray(wd.transpose(0, 1, 4, 3, 2)).reshape(
            EPC, HGS, P, FT * P)
        in_maps.append({"xq": xqc, "wgq": wgc, "wuq": wuc, "wdq": wdc})
    return in_maps


def _numpy_fallback(x, w_gate, w_up, w_down, counts):
    out = np.empty((x.shape[0], w_down.shape[1]), np.float32)
    o = 0
    for e in range(len(counts)):
        n = int(counts[e])
        xi = x[o:o + n]
        gate = xi @ w_gate[e].T
        up = xi @ w_up[e].T
        hidden = (gate / (1.0 + np.exp(-gate))) * up
        out[o:o + n] = hidden @ w_down[e].T
        o += n
    return out


def kernel(x, w_gate, w_up, w_down, tokens_per_expert):
    global LAST_RESULTS
    x = np.asarray(x, dtype=np.float32)
    w_gate = np.asarray(w_gate, dtype=np.float32)
    w_up = np.asarray(w_up, dtype=np.float32)
    w_down = np.asarray(w_down, dtype=np.float32)
    counts = np.asarray(tokens_per_expert).astype(np.int64)

    if not (counts.shape == (E,) and np.all(counts == TPE)):
        # Non-uniform routing: the compiled program is shaped for the
        # uniform split the reference generator produces.
        return _numpy_fallback(x, w_gate, w_up, w_down, counts)

    nc = get_nc()
    res = run_bass_kernel_spmd(
        nc, make_in_maps(x, w_gate, w_up, w_down), list(range(NCORES)),
        trace=TRACE, **TRACE_KW,
    )
    LAST_RESULTS = res
    out = np.empty((T, H), np.float32)
    for c in range(NCORES):
        o = res.results[c]["outq"]  # [EPC, HGS, P, TPE]
        for el in range(EPC):
            t0 = (c * EPC + el) * TPE
            # out[t0+t, g*P+p] = o[el, g, p, t]
            out[t0:t0 + TPE] = o[el].transpose(2, 0, 1).reshape(TPE, H)
    return out


# revision 15
# speedup vs baseline: 1.0068x; 1.0068x over previous
"""Grouped SwiGLU MoE MLP (16 experts) on 8 NeuronCores, expert-parallel.

Reference computation, per expert e over its contiguous token slice xi:
    out = (silu(xi @ w_gate[e].T) * (xi @ w_up[e].T)) @ w_down[e].T

Sharding: expert-parallel. Core c owns experts {2c, 2c+1}; the host hands it
the matching contiguous 2048-token slice of x (tokens are pre-sorted by
expert), so no device-side collectives are needed.

All matmul operands are bf16 (host-cast): the PE streaming rate matches
fp32r but FWL halves LDWEIGHTS time and input DMA bytes halve. PSUM
accumulation stays fp32 and the output is fp32 (~4e-3 rel err, inside the
2e-2 gate).

Layout: pre-packed on the host so every DMA is a contiguous block:
  xq  [EPC, P, HT, TPE]   xq[e,p,a,t]  = x[e*TPE+t, a*P+p]
  wgq [EPC, FT, P, HT*P]  wgq[e,f,p,a*P+j] = w_gate[e, f*P+j, a*P+p]
  wuq  same as wgq for w_up
  wdq [EPC, HG, P, FT*P]  wdq[e,g,p,f*P+j] = w_down[e, g*P+j, f*P+p]
  outq[EPC, HG, P, TPE]   outq[e,g,p,t] = out[e*TPE+t, g*P+p]

Schedule (from trace analysis of earlier revisions):
- The framework preamble ends ~6.5us and the first DMA packet lands ~8us;
  NWARM dummy matmuls on a zeroed tile hold HAM at K=8/8 through that
  window so real work starts at 2.4GHz.
- Chains are t-half-outer so the first f-tile only needs the th=0 half of
  x (1MB instead of 2MB) before the PE can run 32 back-to-back matmuls,
  and so each t-half's PSUM pair drains while the other half's chains run.
- The Scalar queue runs ONLY the silu ACTIVATEs: a dma_start whose
  pool-pacing semaphore isn't yet satisfied parks its whole queue, and
  parking the silu queue delays PSUM consumption and costs the PE one
  matmul slot per group. All steady-state DMA goes on the Sync ring,
  ordered so no paced load ever sits ahead of a store it would block.
- Expert 0's first-tile weights + x are split across both rings for
  startup bandwidth (the Scalar queue is empty until the first silu).
"""

import numpy as np
import ml_dtypes

import concourse.bass as bass
import concourse.bacc as bacc
import concourse.mybir as mybir
from concourse import tile
from concourse.bass_utils import run_bass_kernel_spmd

E, T, H, F = 16, 16384, 1024, 2048
NCORES = 8
EPC = E // NCORES          # experts per core
TPE = T // E               # tokens per expert (uniform fast path)
P = 128                    # SBUF partitions
HT = H // P                # 8 h-tiles (contraction tiles for gate/up)
FT = F // P                # 16 f-tiles
HGS = H // P               # 8 output h-groups for down proj
NT = 512                   # matmul moving free dim (PSUM bank = 512 fp32)
TH = TPE // NT             # 2 t-halves
NWARM = 16                 # dummy matmuls to pre-warm the PE clock

BF16 = mybir.dt.bfloat16
F32 = mybir.dt.float32
BF16_NP = ml_dtypes.bfloat16

_CACHE = {}

# Set by run for test harness introspection (exec_time_ns, profile).
LAST_RESULTS = None
TRACE = False
TRACE_KW = {}


def _build_nc():
    nc = bacc.Bacc()
    xq = nc.dram_tensor("xq", [EPC, P, HT, TPE], BF16, kind="ExternalInput")
    wgq = nc.dram_tensor("wgq", [EPC, FT, P, HT * P], BF16, kind="ExternalInput")
    wuq = nc.dram_tensor("wuq", [EPC, FT, P, HT * P], BF16, kind="ExternalInput")
    wdq = nc.dram_tensor("wdq", [EPC, HGS, P, FT * P], BF16, kind="ExternalInput")
    outq = nc.dram_tensor("outq", [EPC, HGS, P, TPE], F32, kind="ExternalOutput")

    with tile.TileContext(nc) as tc:
        with (
            tc.tile_pool(name="xp", bufs=2) as xp,
            tc.tile_pool(name="wgp", bufs=5) as wgp,
            tc.tile_pool(name="wup", bufs=5) as wup,
            tc.tile_pool(name="wdp", bufs=8) as wdp,
            tc.tile_pool(name="hid", bufs=FT + 1) as hidp,
            tc.tile_pool(name="tmp", bufs=5) as tmpp,
            tc.tile_pool(name="osb", bufs=4) as osbp,
            tc.tile_pool(name="ps", bufs=8, space=bass.MemorySpace.PSUM) as psp,
        ):
            # PE warm-up (see module docstring). The dummy stream must end
            # no earlier than the startup DMA (~14.5us) or the PE idles and
            # HAM re-throttles the first real chains: 16 N=512 dummies burn
            # ~14x427ns cold + 2x216ns warm from ~9us. The memset runs on
            # the DVE, which is idle until the first mul.
            warm = tmpp.tile([P, NT], BF16, tag="tmp", name="warm")
            nc.vector.memset(warm[:], 0.0)
            wps = psp.tile([P, NT], F32, tag="ps", name="warm_ps")
            for _ in range(NWARM):
                nc.tensor.matmul(wps[:], warm[:, 0:P], warm[:],
                                 start=True, stop=True)

            xts = {}
            wgts = {}
            wuts = {}

            def prefetch_head(el):
                """First f-tile weights + x. For expert 0 this is the
                startup-critical burst and is split across both rings; the
                Scalar queue is otherwise empty until the first silu."""
                xt = xp.tile([P, HT, TPE], BF16, tag="xt", name=f"x{el}")
                xts[el] = xt
                wgt = wgp.tile([P, HT * P], BF16, tag="wg", name=f"wg{el}_0")
                nc.sync.dma_start(wgt[:], wgq[el, 0])
                wgts[(el, 0)] = wgt
                # x's th=0 half before wu: the up-chain consumes wu only
                # 8 matmuls after the gate chain starts, but every chain
                # needs x.
                nc.scalar.dma_start(xt[:, 4:8, 0:NT], xq[el][:, 4:8, 0:NT])
                wut = wup.tile([P, HT * P], BF16, tag="wu", name=f"wu{el}_0")
                nc.scalar.dma_start(wut[:], wuq[el, 0])
                wuts[(el, 0)] = wut
                nc.sync.dma_start(xt[:, 0:4, 0:NT], xq[el][:, 0:4, 0:NT])
                nc.sync.dma_start(xt[:, 0:4, NT:TPE], xq[el][:, 0:4, NT:TPE])
                nc.scalar.dma_start(xt[:, 4:8, NT:TPE], xq[el][:, 4:8, NT:TPE])

            prefetch_head(0)
            for el in range(EPC):
                xt = xts[el]
                for ft in range(1, FT):
                    wgt = wgp.tile([P, HT * P], BF16, tag="wg", name=f"wg{el}_{ft}")
                    nc.sync.dma_start(wgt[:], wgq[el, ft])
                    wgts[(el, ft)] = wgt
                    wut = wup.tile([P, HT * P], BF16, tag="wu", name=f"wu{el}_{ft}")
                    nc.sync.dma_start(wut[:], wuq[el, ft])
                    wuts[(el, ft)] = wut
                wdts = {}
                for hg in range(HGS):
                    wdt = wdp.tile([P, FT * P], BF16, tag="wd", name=f"wd{el}_{hg}")
                    nc.sync.dma_start(wdt[:], wdq[el, hg])
                    wdts[hg] = wdt

                # Gate/up, t-half-outer: for each f-tile, run the gate and
                # up chains for th=0 (16 MMs), whose silu+mul drain while
                # the th=1 chains (16 MMs) run.
                hids = []
                for ft in range(FT):
                    wgt, wut = wgts.pop((el, ft)), wuts.pop((el, ft))
                    hid = hidp.tile([P, TPE], BF16, tag="hid", name=f"hid{el}_{ft}")
                    for th in range(TH):
                        tsl = slice(th * NT, (th + 1) * NT)
                        g_ps = psp.tile([P, NT], F32, tag="ps", name=f"g{el}_{ft}_{th}")
                        u_ps = psp.tile([P, NT], F32, tag="ps", name=f"u{el}_{ft}_{th}")
                        for ht in range(HT):
                            nc.tensor.matmul(
                                g_ps[:], wgt[:, ht * P:(ht + 1) * P],
                                xt[:, ht, tsl],
                                start=(ht == 0), stop=(ht == HT - 1),
                            )
                        for ht in range(HT):
                            nc.tensor.matmul(
                                u_ps[:], wut[:, ht * P:(ht + 1) * P],
                                xt[:, ht, tsl],
                                start=(ht == 0), stop=(ht == HT - 1),
                            )
                        tmp = tmpp.tile([P, NT], BF16, tag="tmp")
                        nc.scalar.activation(
                            tmp[:], g_ps[:],
                            mybir.ActivationFunctionType.Silu,
                        )
                        nc.vector.tensor_mul(hid[:, tsl], tmp[:], u_ps[:])
                    hids.append(hid)

                # Next expert's head prefetch goes out before this expert's
                # output stores so its x/weights are resident at the
                # expert boundary.
                if el + 1 < EPC:
                    prefetch_head(el + 1)

                # Down projection, t-half-outer: th=0's 16-MM chain
                # completes before th=1's starts, so its copy+store overlap
                # the th=1 chain and the post-loop tail is one store deep.
                for hg in range(HGS):
                    wdt = wdts[hg]
                    for th in range(TH):
                        tsl = slice(th * NT, (th + 1) * NT)
                        if el == EPC - 1 and hg == HGS - 1 and th == TH - 1:
                            # Final t-half: two N=256 chains into SEPARATE
                            # PSUM banks (a shared bank would make the first
                            # chain's copy collide with the second chain's
                            # writes), so the first quarter's store overlaps
                            # the second chain and the post-loop tail is one
                            # 64KB-store deep per queue.
                            NQ = NT // 2
                            d_qs = [psp.tile([P, NT], F32, tag="ps",
                                             name=f"d{el}_{hg}_{th}_{q}")
                                    for q in range(2)]
                            osb = osbp.tile([P, NT], F32, tag="osb")
                            for q in range(2):
                                qsl = slice(q * NQ, (q + 1) * NQ)
                                for ft in range(FT):
                                    nc.tensor.matmul(
                                        d_qs[q][:, 0:NQ],
                                        wdt[:, ft * P:(ft + 1) * P],
                                        hids[ft][:, th * NT + q * NQ:
                                                  th * NT + (q + 1) * NQ],
                                        start=(ft == 0), stop=(ft == FT - 1),
                                    )
                                if q == 0:
                                    nc.vector.tensor_copy(osb[:, qsl],
                                                          d_qs[0][:, 0:NQ])
                                    nc.scalar.dma_start(
                                        outq[el, hg][:, th * NT:th * NT + NQ],
                                        osb[:, qsl])
                            nc.vector.tensor_copy(osb[:, NQ:NQ + NQ // 2],
                                                  d_qs[1][:, 0:NQ // 2])
                            nc.scalar.activation(
                                osb[:, NQ + NQ // 2:NT],
                                d_qs[1][:, NQ // 2:NQ],
                                mybir.ActivationFunctionType.Copy,
                            )
                            nc.sync.dma_start(
                                outq[el, hg][:, th * NT + NQ:
                                             th * NT + NQ + NQ // 2],
                                osb[:, NQ:NQ + NQ // 2])
                            nc.scalar.dma_start(
                                outq[el, hg][:, th * NT + NQ + NQ // 2:
                                             (th + 1) * NT],
                                osb[:, NQ + NQ // 2:NT])
                        else:
                            d_ps = psp.tile([P, NT], F32, tag="ps",
                                            name=f"d{el}_{hg}_{th}")
                            for ft in range(FT):
                                nc.tensor.matmul(
                                    d_ps[:], wdt[:, ft * P:(ft + 1) * P],
                                    hids[ft][:, tsl],
                                    start=(ft == 0), stop=(ft == FT - 1),
                                )
                            osb = osbp.tile([P, NT], F32, tag="osb")
                            nc.vector.tensor_copy(osb[:], d_ps[:])
                            nc.sync.dma_start(outq[el, hg][:, tsl], osb[:])
    return nc


def get_nc():
    if "nc" not in _CACHE:
        nc = _build_nc()
        nc.finalize()
        _CACHE["nc"] = nc
    return _CACHE["nc"]


def make_in_maps(x, w_gate, w_up, w_down):
    xb = x.astype(BF16_NP)
    wgb = w_gate.astype(BF16_NP)
    wub = w_up.astype(BF16_NP)
    wdb = w_down.astype(BF16_NP)
    in_maps = []
    for c in range(NCORES):
        e0 = c * EPC
        # xq[e,p,a,t] = x[e*TPE+t, a*P+p]
        xs = xb[e0 * TPE:(e0 + EPC) * TPE].reshape(EPC, TPE, HT, P)
        xqc = np.ascontiguousarray(xs.transpose(0, 3, 2, 1))
        # wgq[e,f,p,a*P+j] = w_gate[e, f*P+j, a*P+p]
        wg = wgb[e0:e0 + EPC].reshape(EPC, FT, P, HT, P)
        wgc = np.ascontiguousarray(wg.transpose(0, 1, 4, 3, 2)).reshape(
            EPC, FT, P, HT * P)
        wu = wub[e0:e0 + EPC].reshape(EPC, FT, P, HT, P)
        wuc = np.ascontiguousarray(wu.transpose(0, 1, 4, 3, 2)).reshape(
            EPC, FT, P, HT * P)
        # wdq[e,g,p,f*P+j] = w_down[e, g*P+j, f*P+p]
        wd = wdb[e0:e0 + EPC].reshape(EPC, HGS, P, FT, P)
        wdc = np.ascontiguousarray(wd.transpose(0, 1, 4, 3, 2)).reshape(
            EPC, HGS, P, FT * P)
        in_maps.append({"xq": xqc, "wgq": wgc, "wuq": wuc, "wdq": wdc})
    return in_maps


def _numpy_fallback(x, w_gate, w_up, w_down, counts):
    out = np.empty((x.shape[0], w_down.shape[1]), np.float32)
    o = 0
    for e in range(len(counts)):
        n = int(counts[e])
        xi = x[o:o + n]
        gate = xi @ w_gate[e].T
        up = xi @ w_up[e].T
        hidden = (gate / (1.0 + np.exp(-gate))) * up
        out[o:o + n] = hidden @ w_down[e].T
        o += n
    return out


def kernel(x, w_gate, w_up, w_down, tokens_per_expert):
    global LAST_RESULTS
    x = np.asarray(x, dtype=np.float32)
    w_gate = np.asarray(w_gate, dtype=np.float32)
    w_up = np.asarray(w_up, dtype=np.float32)
    w_down = np.asarray(w_down, dtype=np.float32)
    counts = np.asarray(tokens_per_expert).astype(np.int64)

    if not (counts.shape == (E,) and np.all(counts == TPE)):
        # Non-uniform routing: the compiled program is shaped for the
        # uniform split the reference generator produces.
        return _numpy_fallback(x, w_gate, w_up, w_down, counts)

    nc = get_nc()
    res = run_bass_kernel_spmd(
        nc, make_in_maps(x, w_gate, w_up, w_down), list(range(NCORES)),
        trace=TRACE, **TRACE_KW,
    )
    LAST_RESULTS = res
    out = np.empty((T, H), np.float32)
    for c in range(NCORES):
        o = res.results[c]["outq"]  # [EPC, HGS, P, TPE]
        for el in range(EPC):
            t0 = (c * EPC + el) * TPE
            # out[t0+t, g*P+p] = o[el, g, p, t]
            out[t0:t0 + TPE] = o[el].transpose(2, 0, 1).reshape(TPE, H)
    return out
